# revision 3
# baseline (speedup 1.0000x reference)
"""GAT 2-layer kernel for 8 TRN2 NeuronCores — single-launch version.

Strategy (edge-parallel per sharding hint): destination nodes are split
into 8 contiguous slices (6250/core). Each core owns all edges into its
slice, sorted by dst and packed into a uniform [NB x TB] grid of
128-edge tiles (identical program on all cores).

One launch does everything:
  phase 0:  each core projects its own x-slice (f16 in, f32 accum) ->
            t12 rows [h1 | alpha_src1 | alpha_dst1]; AllGather.
  layer 1:  per edge tile: indirect-gather rows by src (h|asrc) and the
            adst column by dst; w = exp(leaky_relu(asrc+adst)) (no max
            subtraction needed: exponents are O(1)); selection matrix
            S[e,n] = (dst_local==n); one PSUM matmul per tile
            accumulates [S^T @ (w*h) | S^T @ w]. Per dst block: divide,
            bias, ELU, and the layer-2 projection fused -> t3 rows
            [h2 | asrc2 | adst2]; AllGather.
  layer 2:  same edge phase on t3; divide + bias -> output slice (f16).

Vector work is batched G=16 tiles per instruction via strided views;
only the gathers and the per-tile matmul remain per-tile. Transfers are
shrunk (f16 x, u16 indices, u8 dst-locals, f16 output) and overlapped
with host-side edge prep via async device_put. Bass + NEFF compilation
and a warm run happen at import time (shapes are static); kernel() only
preps grids, transfers, executes, and unpacks.
"""
import sys

sys.path.insert(0, '/opt/trn_rl_repo')

import numpy as np

import concourse.bass as bass
import concourse.bacc as bacc
import concourse.mybir as mybir
import concourse.tile as tile
from concourse.vector_clock import ScopedClock

f32 = mybir.dt.float32
f16 = mybir.dt.float16
i32 = mybir.dt.int32
u16 = mybir.dt.uint16
u8 = mybir.dt.uint8
P = 128
NCORES = 8
NEG_SLOPE = 0.2
EPS = 1e-16
HEADS1, OUT1 = 4, 32
HEADS2, OUT2 = 1, 32
F_IN = 128
F1 = HEADS1 * OUT1          # 128
N = 50000
NPC = N // NCORES           # 6250 nodes per core
NB = (NPC + P - 1) // P     # 49 dst blocks per core
PADN = NB * P               # 6272 padded nodes per core
TBLN = NCORES * PADN        # 50176 table rows
ROW1 = F_IN + 2 * HEADS1    # 136: [h1 | asrc1 | adst1]
ROW2 = OUT2 + 2 * HEADS2    # 34:  [h2 | asrc2 | adst2]
G = 16                      # tiles per batch group
TB_DEFAULT = 35             # padded tiles per dst block (rebuilt if exceeded)
DLOC_PAD = 255              # u8 pad: never equals a node index 0..127

_MAX_WAITS = 1


def _split_excess_waits(nc, max_waits=_MAX_WAITS):
    # this walrus build rejects >1 sem-wait per instruction; hoist excess
    # waits onto same-engine nops inserted right before the instruction
    for bb in nc.main_func.blocks:
        lst = bb.instructions
        out = []
        for inst in lst:
            si = inst.sync_info
            waits = list(si.on_wait) if si is not None and si.on_wait else []
            if len(waits) > max_waits:
                excess, keep = waits[:-max_waits], waits[-max_waits:]
                for w in excess:
                    nop = mybir.InstNoOp(
                        name=nc.get_next_instruction_name(), ins=[], outs=[]
                    )
                    nop.engine = inst.engine
                    nop.sync_info = mybir.SyncInfo(on_wait=[w], on_update=[])
                    nc.register_instruction(nop)
                    out.append(nop)
                si.on_wait.clear()
                for w in keep:
                    si.on_wait.append(w)
            out.append(inst)
        lst.clear()
        lst.extend(out)


def _patched_drain_and_barrier(self, tick_clock, wait_clock):
    nc = self.nc
    drain_inst = nc.sync.drain()
    wait_clock.add_sem_waits(
        drain_inst.ins, ScopedClock({None: tick_clock.global_clock})
    )
    nc.all_engine_barrier()
    assert self.sems is not None
    popped = nc._tile_sem_poison_stack.pop()
    assert popped is self._sem_poison
    nc.clear_and_free_semaphores(list(self.sems.allocated().values()))
    nc.all_engine_barrier()


tile.TileContext._drain_and_barrier = _patched_drain_and_barrier


def _v(ap_base, off, dims):
    """Strided view of a tile: partition dim kept, free dims replaced."""
    return bass.AP(ap_base.tensor, ap_base.offset + off, [ap_base.ap[0]] + dims)


def _edge_phase(nc, pools, table, rowlen, fdim, nheads, srcg, dstg, dlocg,
                iota_t, ngroups, tb, out_cb):
    """Edge aggregation: per tile one row-gather by src, one adst-column
    gather by dst, one PSUM matmul; vector work batched per G tiles.
    table rows: [feat(fdim) | asrc(nheads) | adst(nheads)].
    out_cb(b, acc) consumes each finished block; acc = [S^T(w*h) | S^T w].
    """
    pool, psum = pools
    H = nheads
    C = fdim // H
    MR = fdim + H  # matmul rhs width per tile: [m | w]
    ntiles = NB * tb
    acc = None
    for q in range(ngroups):
        ixs_u = pool.tile([P, G], u16, tag="ixsu")
        nc.sync.dma_start(out=ixs_u[:], in_=srcg[q])
        ixd_u = pool.tile([P, G], u16, tag="ixdu")
        nc.sync.dma_start(out=ixd_u[:], in_=dstg[q])
        dloc_u = pool.tile([P, G], u8, tag="dlocu")
        nc.sync.dma_start(out=dloc_u[:], in_=dlocg[q])
        ixs = pool.tile([P, G], i32, tag="ixs")
        nc.vector.tensor_copy(out=ixs[:], in_=ixs_u[:])
        ixd = pool.tile([P, G], i32, tag="ixd")
        nc.vector.tensor_copy(out=ixd[:], in_=ixd_u[:])
        dloc = pool.tile([P, G], f32, tag="dloc")
        nc.vector.tensor_copy(out=dloc[:], in_=dloc_u[:])

        nt = min(G, ntiles - q * G)  # live tiles in this group
        if nt <= 0:
            continue
        gs = pool.tile([P, G * rowlen], f32, tag="gs")
        ad = pool.tile([P, G * H], f32, tag="ad")
        for t in range(nt):
            nc.gpsimd.indirect_dma_start(
                out=gs[:, t * rowlen:(t + 1) * rowlen], out_offset=None,
                in_=table[:],
                in_offset=bass.IndirectOffsetOnAxis(ap=ixs[:, t:t + 1], axis=0))
            nc.gpsimd.indirect_dma_start(
                out=ad[:, t * H:(t + 1) * H], out_offset=None, in_=table[:],
                in_offset=bass.IndirectOffsetOnAxis(ap=ixd[:, t:t + 1], axis=0),
                element_offset=fdim + H)

        # S[e, g, n] = (iota[n] == dloc[e, g])   [P, G*P]
        s_all = pool.tile([P, G * P], f32, tag="sall")
        nc.vector.tensor_tensor(
            out=_v(s_all[:], 0, [[P, G], [1, P]]),
            in0=_v(iota_t[:], 0, [[0, G], [1, P]]),
            in1=_v(dloc[:], 0, [[1, G], [0, P]]),
            op=mybir.AluOpType.is_equal)

        # w = exp(leaky_relu(asrc[src] + adst[dst]))   [P, G*H] contiguous
        w_c = pool.tile([P, G * H], f32, tag="wc")
        nc.vector.tensor_tensor(
            out=_v(w_c[:], 0, [[H, G], [1, H]]),
            in0=_v(gs[:], fdim, [[rowlen, G], [1, H]]),
            in1=_v(ad[:], 0, [[H, G], [1, H]]),
            op=mybir.AluOpType.add)
        lr = pool.tile([P, G * H], f32, tag="lr")
        nc.vector.tensor_scalar(out=lr[:], in0=w_c[:], scalar1=NEG_SLOPE,
                                scalar2=None, op0=mybir.AluOpType.mult)
        nc.vector.tensor_tensor(out=w_c[:], in0=w_c[:], in1=lr[:],
                                op=mybir.AluOpType.max)
        nc.scalar.activation(w_c[:], w_c[:], mybir.ActivationFunctionType.Exp)

        # m_all per tile: [w*h (fdim) | w (H)]   [P, G*MR]
        m_all = pool.tile([P, G * MR], f32, tag="mall")
        nc.vector.tensor_copy(
            out=_v(m_all[:], fdim, [[MR, G], [1, H]]),
            in_=_v(w_c[:], 0, [[H, G], [1, H]]))
        nc.vector.tensor_tensor(
            out=_v(m_all[:], 0, [[MR, G], [C, H], [1, C]]),
            in0=_v(gs[:], 0, [[rowlen, G], [C, H], [1, C]]),
            in1=_v(w_c[:], 0, [[H, G], [1, H], [0, C]]),
            op=mybir.AluOpType.mult)

        for t in range(nt):
            gtile = q * G + t
            tt = gtile % tb
            if tt == 0:
                acc = psum.tile([P, MR], f32, space="PSUM", tag="acc")
            nc.tensor.matmul(acc[:], lhsT=s_all[:, t * P:(t + 1) * P],
                             rhs=m_all[:, t * MR:(t + 1) * MR],
                             start=(tt == 0), stop=(tt == tb - 1))
            if tt == tb - 1:
                out_cb(gtile // tb, acc)


def _build_kernel(TB, NGRP):
    nc = bacc.Bacc(None, target_bir_lowering=False)
    xT = nc.dram_tensor("xT", [F_IN, PADN], f16, kind="ExternalInput")
    w1cat = nc.dram_tensor("w1cat", [F_IN, ROW1], f16, kind="ExternalInput")
    w2cat = nc.dram_tensor("w2cat", [F1, ROW2], f32, kind="ExternalInput")
    b1t = nc.dram_tensor("b1t", [P, F1], f32, kind="ExternalInput")
    b2t = nc.dram_tensor("b2t", [P, OUT2], f32, kind="ExternalInput")
    iota = nc.dram_tensor("iota", [P, P], f32, kind="ExternalInput")
    ident = nc.dram_tensor("ident", [P, P], f32, kind="ExternalInput")
    srcg = nc.dram_tensor("srcg", [NGRP, P, G], u16, kind="ExternalInput")
    dstg = nc.dram_tensor("dstg", [NGRP, P, G], u16, kind="ExternalInput")
    dlocg = nc.dram_tensor("dlocg", [NGRP, P, G], u8, kind="ExternalInput")
    oout = nc.dram_tensor("oout", [PADN, OUT2], f16, kind="ExternalOutput")

    with tile.TileContext(nc) as tc:
        with (
            tc.tile_pool(name="const", bufs=1) as cpool,
            tc.tile_pool(name="sbuf", bufs=3) as pool,
            tc.tile_pool(name="psum", bufs=2, space="PSUM") as psum,
            tc.tile_pool(name="dram", bufs=1, space="DRAM") as dram,
        ):
            w1_t = cpool.tile([F_IN, ROW1], f16)
            nc.sync.dma_start(out=w1_t[:], in_=w1cat[:])
            w2_t = cpool.tile([F1, ROW2], f32)
            nc.sync.dma_start(out=w2_t[:], in_=w2cat[:])
            b1_t = cpool.tile([P, F1], f32)
            nc.sync.dma_start(out=b1_t[:], in_=b1t[:])
            b2_t = cpool.tile([P, OUT2], f32)
            nc.sync.dma_start(out=b2_t[:], in_=b2t[:])
            iota_t = cpool.tile([P, P], f32)
            nc.sync.dma_start(out=iota_t[:], in_=iota[:])
            ident_t = cpool.tile([P, P], f32)
            nc.sync.dma_start(out=ident_t[:], in_=ident[:])

            t12c = dram.tile([PADN, ROW1], f32)
            t12f = dram.tile([TBLN, ROW1], f32)
            t3c = dram.tile([PADN, ROW2], f32)
            t3f = dram.tile([TBLN, ROW2], f32)

            # ---- phase 0: own slice of t12 = [x@W1 | x@W1 A1s | x@W1 A1d]
            for i in range(NB):
                xTt = pool.tile([F_IN, P], f16, tag="xTt")
                nc.sync.dma_start(out=xTt[:], in_=xT[:, i * P:(i + 1) * P])
                h_ps = psum.tile([P, ROW1], f32, space="PSUM", tag="mmp", bufs=1)
                nc.tensor.matmul(h_ps[:], lhsT=xTt[:], rhs=w1_t[:],
                                 start=True, stop=True)
                h_sb = pool.tile([P, ROW1], f32, tag="hsb")
                nc.vector.tensor_copy(out=h_sb[:], in_=h_ps[:])
                nc.sync.dma_start(out=t12c[:][i * P:(i + 1) * P, :], in_=h_sb[:])

            nc.gpsimd.collective_compute(
                "AllGather", mybir.AluOpType.bypass,
                replica_groups=[list(range(NCORES))],
                ins=[t12c.opt()], outs=[t12f.opt()])

            # ---- layer 1 edge phase; epilogue fuses ELU + layer-2 projection
            def epi1(b, acc):
                r = pool.tile([P, HEADS1], f32, tag="r")
                nc.vector.tensor_scalar(out=r[:], in0=acc[:, F1:F1 + HEADS1],
                                        scalar1=EPS, scalar2=None,
                                        op0=mybir.AluOpType.add)
                nc.vector.reciprocal(out=r[:], in_=r[:])
                o = pool.tile([P, F1], f32, tag="o")
                nc.vector.tensor_tensor(
                    out=_v(o[:], 0, [[OUT1, HEADS1], [1, OUT1]]),
                    in0=_v(acc[:], 0, [[OUT1, HEADS1], [1, OUT1]]),
                    in1=_v(r[:], 0, [[1, HEADS1], [0, OUT1]]),
                    op=mybir.AluOpType.mult)
                nc.vector.tensor_tensor(out=o[:], in0=o[:], in1=b1_t[:],
                                        op=mybir.AluOpType.add)
                # elu(o) = max(o,0) + exp(min(o,0)) - 1
                mn = pool.tile([P, F1], f32, tag="mn")
                nc.vector.tensor_scalar(out=mn[:], in0=o[:], scalar1=0.0,
                                        scalar2=None, op0=mybir.AluOpType.min)
                nc.scalar.activation(mn[:], mn[:],
                                     mybir.ActivationFunctionType.Exp)
                nc.vector.tensor_scalar(out=o[:], in0=o[:], scalar1=0.0,
                                        scalar2=None, op0=mybir.AluOpType.max)
                nc.vector.tensor_tensor(out=o[:], in0=o[:], in1=mn[:],
                                        op=mybir.AluOpType.add)
                nc.vector.tensor_scalar(out=o[:], in0=o[:], scalar1=-1.0,
                                        scalar2=None, op0=mybir.AluOpType.add)
                # t3 rows = elu_out @ [W2 | W2 a2s | W2 a2d]
                oT_ps = psum.tile([P, P], f32, space="PSUM", tag="T", bufs=1)
                nc.tensor.transpose(out=oT_ps[:], in_=o[:], identity=ident_t[:])
                oT = pool.tile([P, F1], f32, tag="oT")
                nc.vector.tensor_copy(out=oT[:], in_=oT_ps[:])
                t3_ps = psum.tile([P, ROW2], f32, space="PSUM", tag="mmp", bufs=1)
                nc.tensor.matmul(t3_ps[:], lhsT=oT[:], rhs=w2_t[:],
                                 start=True, stop=True)
                t3_sb = pool.tile([P, ROW2], f32, tag="t3s")
                nc.vector.tensor_copy(out=t3_sb[:], in_=t3_ps[:])
                nc.sync.dma_start(out=t3c[:][b * P:(b + 1) * P, :], in_=t3_sb[:])

            _edge_phase(nc, (pool, psum), t12f, ROW1, F1, HEADS1,
                        srcg, dstg, dlocg, iota_t, NGRP, TB, epi1)

            nc.gpsimd.collective_compute(
                "AllGather", mybir.AluOpType.bypass,
                replica_groups=[list(range(NCORES))],
                ins=[t3c.opt()], outs=[t3f.opt()])

            # ---- layer 2 edge phase
            def epi2(b, acc):
                r2 = pool.tile([P, 1], f32, tag="r2")
                nc.vector.tensor_scalar(out=r2[:], in0=acc[:, OUT2:OUT2 + 1],
                                        scalar1=EPS, scalar2=None,
                                        op0=mybir.AluOpType.add)
                nc.vector.reciprocal(out=r2[:], in_=r2[:])
                o2 = pool.tile([P, OUT2], f32, tag="o2")
                nc.vector.tensor_tensor(out=o2[:], in0=acc[:, 0:OUT2],
                                        in1=r2[:, 0:1].to_broadcast([P, OUT2]),
                                        op=mybir.AluOpType.mult)
                nc.vector.tensor_tensor(out=o2[:], in0=o2[:], in1=b2_t[:],
                                        op=mybir.AluOpType.add)
                o2h = pool.tile([P, OUT2], f16, tag="o2h")
                nc.vector.tensor_copy(out=o2h[:], in_=o2[:])
                nc.sync.dma_start(out=oout[b * P:(b + 1) * P, :], in_=o2h[:])

            _edge_phase(nc, (pool, psum), t3f, ROW2, OUT2, HEADS2,
                        srcg, dstg, dlocg, iota_t, NGRP, TB, epi2)

    nc.compile()
    _split_excess_waits(nc)
    return nc


# ---------------------------------------------------------------------------
# launcher: AOT-compile the PJRT wrapper once, reuse across calls

def _make_runner(nc):
    import jax
    from jax.sharding import Mesh, PartitionSpec
    from jax.experimental.shard_map import shard_map
    from concourse.bass2jax import (install_neuronx_cc_hook, _bass_exec_p,
                                    partition_id_tensor)

    install_neuronx_cc_hook()
    partition_name = nc.partition_id_tensor.name if nc.partition_id_tensor else None
    in_names, out_names, out_avals = [], [], []
    for alloc in nc.m.functions[0].allocations:
        if not isinstance(alloc, mybir.MemoryLocationSet):
            continue
        name = alloc.memorylocations[0].name
        if alloc.kind == "ExternalInput":
            if name != partition_name:
                in_names.append(name)
        elif alloc.kind == "ExternalOutput":
            out_names.append(name)
            out_avals.append(jax.core.ShapedArray(
                tuple(alloc.tensor_shape), mybir.dt.np(alloc.dtype)))
    n_params = len(in_names)
    all_names = list(in_names) + list(out_names)
    if partition_name is not None:
        all_names.append(partition_name)
    donate = tuple(range(n_params, n_params + len(out_names)))

    def _body(*args):
        operands = list(args)
        if partition_name is not None:
            operands.append(partition_id_tensor())
        return tuple(_bass_exec_p.bind(
            *operands, out_avals=tuple(out_avals), in_names=tuple(all_names),
            out_names=tuple(out_names), lowering_input_output_aliases=(),
            sim_require_finite=True, sim_require_nnan=True, nc=nc))

    devices = jax.devices()[:NCORES]
    mesh = Mesh(np.asarray(devices), ("core",))
    nio = n_params + len(out_names)
    sharded = jax.jit(
        shard_map(_body, mesh=mesh, in_specs=(PartitionSpec("core"),) * nio,
                  out_specs=(PartitionSpec("core"),) * len(out_names),
                  check_rep=False),
        donate_argnums=donate, keep_unused=True)
    in_structs = []
    for alloc in nc.m.functions[0].allocations:
        if not isinstance(alloc, mybir.MemoryLocationSet):
            continue
        if alloc.memorylocations[0].name in in_names:
            shp = tuple(alloc.tensor_shape)
            in_structs.append(jax.ShapeDtypeStruct(
                (NCORES * shp[0],) + shp[1:], mybir.dt.np(alloc.dtype)))
    out_structs = [jax.ShapeDtypeStruct((NCORES * a.shape[0],) + a.shape[1:],
                                        a.dtype) for a in out_avals]
    compiled = sharded.lower(*in_structs, *out_structs).compile()
    return {
        "compiled": compiled,
        "in_names": in_names,
        "out_names": out_names,
        "sharding": compiled.input_shardings[0][0],
        "out_structs": [(tuple(s.shape), s.dtype) for s in out_structs],
        "in_structs": [(tuple(s.shape), s.dtype) for s in in_structs],
    }


_RUNNERS = {}


def _get_runner(TB, NGRP, warm=False):
    key = (TB, NGRP)
    if key not in _RUNNERS:
        nc = _build_kernel(TB, NGRP)
        runner = _make_runner(nc)
        if warm:
            import jax
            ins = [np.zeros(s, d) for s, d in runner["in_structs"]]
            outs = [np.zeros(s, d) for s, d in runner["out_structs"]]
            jax.block_until_ready(runner["compiled"](*ins, *outs))
        _RUNNERS[key] = runner
    return _RUNNERS[key]


# ---------------------------------------------------------------------------
# host-side edge prep (vectorized)

def _prep_edges(src, dst, TB_hint):
    """Pack edges into per-core [NGRP, P, G] grids (concatenated on axis 0),
    writing the device layout directly. Returns (srcg, dstg, dlocg, TB, NGRP).
    """
    order = np.argsort(dst, kind='stable')
    s = src[order]
    d = dst[order]
    ci = d // NPC                      # owning core (contiguous after sort)
    ld = d - ci * NPC                  # local dst within core slice
    blk_l = ld // P
    blk_g = ci * NB + blk_l
    cnt = np.bincount(blk_g, minlength=NCORES * NB)
    TB = max(int(-(-cnt.max() // P)), 1, TB_hint)
    starts = np.zeros(NCORES * NB, np.int64)
    np.cumsum(cnt[:-1], out=starts[1:])
    rank = np.arange(len(d), dtype=np.int64) - starts[blk_g]
    gtile = blk_l * TB + rank // P     # tile id within core grid
    part = rank % P                    # partition (edge slot within tile)
    q = gtile // G
    tg = gtile - q * G
    ntiles = NB * TB
    NGRP = -(-ntiles // G)
    nslots = NGRP * P * G
    flat = (ci * nslots + q * (P * G) + part * G + tg).astype(np.int64)
    srcg = np.zeros(NCORES * nslots, np.uint16)
    dstg = np.zeros(NCORES * nslots, np.uint16)
    dlocg = np.full(NCORES * nslots, DLOC_PAD, np.uint8)
    srcg[flat] = ((s // NPC) * PADN + s % NPC).astype(np.uint16)
    dstg[flat] = (ci * PADN + ld).astype(np.uint16)
    dlocg[flat] = (ld % P).astype(np.uint8)
    shp = (NCORES * NGRP, P, G)
    return srcg.reshape(shp), dstg.reshape(shp), dlocg.reshape(shp), TB, NGRP


def kernel(x, edge_index, W1, a_src1, a_dst1, b1, W2, a_src2, a_dst2, b2):
    import jax
    x = np.asarray(x, np.float32)
    assert x.shape == (N, F_IN), f"unexpected x shape {x.shape}"
    default = _RUNNERS.get((TB_DEFAULT, -(-(NB * TB_DEFAULT) // G)))

    # pack x + weights first and start their transfers (overlaps edge prep)
    xT = np.zeros((NCORES, F_IN, PADN), np.float16)
    for k in range(NCORES):
        xT[k, :, :NPC] = x[k * NPC:(k + 1) * NPC].T
    xT = xT.reshape(NCORES * F_IN, PADN)
    W1 = np.asarray(W1, np.float32)
    A1s = np.zeros((F1, HEADS1), np.float32)
    A1d = np.zeros((F1, HEADS1), np.float32)
    for h in range(HEADS1):
        A1s[h * OUT1:(h + 1) * OUT1, h] = np.asarray(a_src1, np.float32)[h]
        A1d[h * OUT1:(h + 1) * OUT1, h] = np.asarray(a_dst1, np.float32)[h]
    w1cat = np.concatenate([W1, W1 @ A1s, W1 @ A1d], axis=1)  # [F_IN, 136]
    W2 = np.asarray(W2, np.float32)
    w2cat = np.concatenate(
        [W2, W2 @ np.asarray(a_src2, np.float32).reshape(OUT2, 1),
         W2 @ np.asarray(a_dst2, np.float32).reshape(OUT2, 1)], axis=1)

    def rep(a):  # replicate a per-core constant 8x along axis 0
        return np.tile(a, (NCORES,) + (1,) * (a.ndim - 1))

    arrays = {
        "xT": xT,
        "w1cat": rep(w1cat.astype(np.float16)),
        "w2cat": rep(w2cat),
        "b1t": rep(np.tile(np.asarray(b1, np.float32)[None, :], (P, 1))),
        "b2t": rep(np.tile(np.asarray(b2, np.float32)[None, :], (P, 1))),
        "iota": rep(np.tile(np.arange(P, dtype=np.float32)[None, :], (P, 1))),
        "ident": rep(np.eye(P, dtype=np.float32)),
    }
    if default is not None:
        sh = default["sharding"]
        arrays = {k: jax.device_put(v, sh) for k, v in arrays.items()}
        outs = [jax.device_put(np.zeros(s, d), sh)
                for s, d in default["out_structs"]]

    # edge prep on CPU while the above transfers stream in
    loops = np.arange(N, dtype=np.int64)
    src = np.concatenate([np.asarray(edge_index[0], np.int64), loops])
    dst = np.concatenate([np.asarray(edge_index[1], np.int64), loops])
    srcg, dstg, dlocg, TB, NGRP = _prep_edges(src, dst, TB_DEFAULT)
    runner = _get_runner(TB, NGRP)
    if runner is not default:
        arrays = {k: np.asarray(v) for k, v in arrays.items()}
        outs = [np.zeros(s, d) for s, d in runner["out_structs"]]
    arrays.update({"srcg": srcg, "dstg": dstg, "dlocg": dlocg})

    ins = [arrays[nm] for nm in runner["in_names"]]
    res = runner["compiled"](*ins, *outs)
    oidx = runner["out_names"].index("oout")
    oo = np.asarray(res[oidx]).astype(np.float32).reshape(NCORES, PADN, OUT2)
    return np.ascontiguousarray(oo[:, :NPC, :].reshape(N, OUT2))


# precompile + warm at import (shapes are static for this problem)
_DEFAULT_NGRP = -(-(NB * TB_DEFAULT) // G)
try:
    _get_runner(TB_DEFAULT, _DEFAULT_NGRP, warm=True)
except Exception:
    _RUNNERS.clear()


# revision 8
# speedup vs baseline: 1.1571x; 1.1571x over previous
"""GAT 2-layer kernel for 8 TRN2 NeuronCores — single-launch version.

Strategy (edge-parallel per sharding hint): destination nodes are split
into 8 contiguous slices (6250/core). Each core owns all edges into its
slice, sorted by dst and packed into a uniform [NB x TB] grid of
128-edge tiles (identical program on all cores).

One launch does everything:
  phase 0:  each core projects its own x-slice (f16 in, f32 accum) ->
            t12 rows [h1 | alpha_src1 | alpha_dst1]; AllGather.
  layer 1:  per edge tile: indirect-gather rows by src (h|asrc) and the
            adst column by dst; w = exp(leaky_relu(asrc+adst)) (no max
            subtraction needed: exponents are O(1)); selection matrix
            S[e,n] = (dst_local==n); one PSUM matmul per tile
            accumulates [S^T @ (w*h) | S^T @ w]. Per dst block: divide,
            bias, ELU, and the layer-2 projection fused -> t3 rows
            [h2 | asrc2 | adst2]; AllGather.
  layer 2:  same edge phase on t3; divide + bias -> output slice (f16).

Vector work is batched G=16 tiles per instruction via strided views;
only the gathers and the per-tile matmul remain per-tile. Transfers are
shrunk (f16 x, u16 indices, u8 dst-locals, f16 output) and overlapped
with host-side edge prep via async device_put. Bass + NEFF compilation
and a warm run happen at import time (shapes are static); kernel() only
preps grids, transfers, executes, and unpacks.
"""
import sys

sys.path.insert(0, '/opt/trn_rl_repo')

import numpy as np

import concourse.bass as bass
import concourse.bacc as bacc
import concourse.mybir as mybir
import concourse.tile as tile
from concourse.vector_clock import ScopedClock

f32 = mybir.dt.float32
f16 = mybir.dt.float16
i32 = mybir.dt.int32
u16 = mybir.dt.uint16
u8 = mybir.dt.uint8
P = 128
NCORES = 8
NEG_SLOPE = 0.2
EPS = 1e-16
HEADS1, OUT1 = 4, 32
HEADS2, OUT2 = 1, 32
F_IN = 128
F1 = HEADS1 * OUT1          # 128
N = 50000
NPC = N // NCORES           # 6250 nodes per core
NB = (NPC + P - 1) // P     # 49 dst blocks per core
PADN = NB * P               # 6272 padded nodes per core
TBLN = NCORES * PADN        # 50176 table rows
ROW1 = F_IN + 2 * HEADS1    # 136: [h1 | asrc1 | adst1]
ROW2 = OUT2 + 2 * HEADS2    # 34:  [h2 | asrc2 | adst2]
G = 16                      # tiles per batch group
TB_DEFAULT = 35             # padded tiles per dst block (rebuilt if exceeded)
DLOC_PAD = 255              # u8 pad: never equals a node index 0..127

_MAX_WAITS = 1


def _split_excess_waits(nc, max_waits=_MAX_WAITS):
    # this walrus build rejects >1 sem-wait per instruction; hoist excess
    # waits onto same-engine nops inserted right before the instruction
    for bb in nc.main_func.blocks:
        lst = bb.instructions
        out = []
        for inst in lst:
            si = inst.sync_info
            waits = list(si.on_wait) if si is not None and si.on_wait else []
            if len(waits) > max_waits:
                excess, keep = waits[:-max_waits], waits[-max_waits:]
                for w in excess:
                    nop = mybir.InstNoOp(
                        name=nc.get_next_instruction_name(), ins=[], outs=[]
                    )
                    nop.engine = inst.engine
                    nop.sync_info = mybir.SyncInfo(on_wait=[w], on_update=[])
                    nc.register_instruction(nop)
                    out.append(nop)
                si.on_wait.clear()
                for w in keep:
                    si.on_wait.append(w)
            out.append(inst)
        lst.clear()
        lst.extend(out)


def _patched_drain_and_barrier(self, tick_clock, wait_clock):
    nc = self.nc
    drain_inst = nc.sync.drain()
    wait_clock.add_sem_waits(
        drain_inst.ins, ScopedClock({None: tick_clock.global_clock})
    )
    nc.all_engine_barrier()
    assert self.sems is not None
    popped = nc._tile_sem_poison_stack.pop()
    assert popped is self._sem_poison
    nc.clear_and_free_semaphores(list(self.sems.allocated().values()))
    nc.all_engine_barrier()


tile.TileContext._drain_and_barrier = _patched_drain_and_barrier


def _v(ap_base, off, dims):
    """Strided view of a tile: partition dim kept, free dims replaced."""
    return bass.AP(ap_base.tensor, ap_base.offset + off, [ap_base.ap[0]] + dims)


def _edge_phase(nc, pools, table, rowlen, fdim, nheads, srcg, dstg, dlocg,
                iota_t, ngroups, tb, out_cb):
    """Edge aggregation: per tile one row-gather by src, one adst-column
    gather by dst, one PSUM matmul; vector work batched per G tiles.
    table rows: [feat(fdim) | asrc(nheads) | adst(nheads)].
    out_cb(b, acc) consumes each finished block; acc = [S^T(w*h) | S^T w].
    """
    pool, psum = pools
    H = nheads
    C = fdim // H
    MR = fdim + H  # matmul rhs width per tile: [m | w]
    ntiles = NB * tb
    acc = None
    for q in range(ngroups):
        ixs_u = pool.tile([P, G], u16, tag="ixsu")
        nc.sync.dma_start(out=ixs_u[:], in_=srcg[q])
        ixd_u = pool.tile([P, G], u16, tag="ixdu")
        nc.sync.dma_start(out=ixd_u[:], in_=dstg[q])
        dloc_u = pool.tile([P, G], u8, tag="dlocu")
        nc.sync.dma_start(out=dloc_u[:], in_=dlocg[q])
        ixs = pool.tile([P, G], i32, tag="ixs")
        nc.vector.tensor_copy(out=ixs[:], in_=ixs_u[:])
        ixd = pool.tile([P, G], i32, tag="ixd")
        nc.vector.tensor_copy(out=ixd[:], in_=ixd_u[:])
        dloc = pool.tile([P, G], f32, tag="dloc")
        nc.vector.tensor_copy(out=dloc[:], in_=dloc_u[:])

        nt = min(G, ntiles - q * G)  # live tiles in this group
        if nt <= 0:
            continue
        gs = pool.tile([P, G * rowlen], f32, tag="gs")
        ad = pool.tile([P, G * H], f32, tag="ad")
        for t in range(nt):
            nc.gpsimd.indirect_dma_start(
                out=gs[:, t * rowlen:(t + 1) * rowlen], out_offset=None,
                in_=table[:],
                in_offset=bass.IndirectOffsetOnAxis(ap=ixs[:, t:t + 1], axis=0))
            nc.gpsimd.indirect_dma_start(
                out=ad[:, t * H:(t + 1) * H], out_offset=None, in_=table[:],
                in_offset=bass.IndirectOffsetOnAxis(ap=ixd[:, t:t + 1], axis=0),
                element_offset=fdim + H)

        # S[e, g, n] = (iota[n] == dloc[e, g])   [P, G*P]
        s_all = pool.tile([P, G * P], f32, tag="sall")
        nc.vector.tensor_tensor(
            out=_v(s_all[:], 0, [[P, G], [1, P]]),
            in0=_v(iota_t[:], 0, [[0, G], [1, P]]),
            in1=_v(dloc[:], 0, [[1, G], [0, P]]),
            op=mybir.AluOpType.is_equal)

        # w = exp(leaky_relu(asrc[src] + adst[dst]))   [P, G*H] contiguous
        w_c = pool.tile([P, G * H], f32, tag="wc")
        nc.vector.tensor_tensor(
            out=_v(w_c[:], 0, [[H, G], [1, H]]),
            in0=_v(gs[:], fdim, [[rowlen, G], [1, H]]),
            in1=_v(ad[:], 0, [[H, G], [1, H]]),
            op=mybir.AluOpType.add)
        lr = pool.tile([P, G * H], f32, tag="lr")
        nc.vector.tensor_scalar(out=lr[:], in0=w_c[:], scalar1=NEG_SLOPE,
                                scalar2=None, op0=mybir.AluOpType.mult)
        nc.vector.tensor_tensor(out=w_c[:], in0=w_c[:], in1=lr[:],
                                op=mybir.AluOpType.max)
        nc.scalar.activation(w_c[:], w_c[:], mybir.ActivationFunctionType.Exp)

        # m_all per tile: [w*h (fdim) | w (H)]   [P, G*MR]
        m_all = pool.tile([P, G * MR], f32, tag="mall")
        nc.vector.tensor_copy(
            out=_v(m_all[:], fdim, [[MR, G], [1, H]]),
            in_=_v(w_c[:], 0, [[H, G], [1, H]]))
        nc.vector.tensor_tensor(
            out=_v(m_all[:], 0, [[MR, G], [C, H], [1, C]]),
            in0=_v(gs[:], 0, [[rowlen, G], [C, H], [1, C]]),
            in1=_v(w_c[:], 0, [[H, G], [1, H], [0, C]]),
            op=mybir.AluOpType.mult)

        for t in range(nt):
            gtile = q * G + t
            tt = gtile % tb
            if tt == 0:
                acc = psum.tile([P, MR], f32, space="PSUM", tag="acc")
            nc.tensor.matmul(acc[:], lhsT=s_all[:, t * P:(t + 1) * P],
                             rhs=m_all[:, t * MR:(t + 1) * MR],
                             start=(tt == 0), stop=(tt == tb - 1))
            if tt == tb - 1:
                out_cb(gtile // tb, acc)


def _build_kernel(TB, NGRP):
    nc = bacc.Bacc(None, target_bir_lowering=False)
    xT = nc.dram_tensor("xT", [F_IN, PADN], f16, kind="ExternalInput")
    w1cat = nc.dram_tensor("w1cat", [F_IN, ROW1], f16, kind="ExternalInput")
    w2cat = nc.dram_tensor("w2cat", [F1, ROW2], f32, kind="ExternalInput")
    b1t = nc.dram_tensor("b1t", [P, F1], f32, kind="ExternalInput")
    b2t = nc.dram_tensor("b2t", [P, OUT2], f32, kind="ExternalInput")
    iota = nc.dram_tensor("iota", [P, P], f32, kind="ExternalInput")
    ident = nc.dram_tensor("ident", [P, P], f32, kind="ExternalInput")
    srcg = nc.dram_tensor("srcg", [NGRP, P, G], u16, kind="ExternalInput")
    dstg = nc.dram_tensor("dstg", [NGRP, P, G], u16, kind="ExternalInput")
    dlocg = nc.dram_tensor("dlocg", [NGRP, P, G], u8, kind="ExternalInput")
    oout = nc.dram_tensor("oout", [PADN, OUT2], f16, kind="ExternalOutput")

    with tile.TileContext(nc) as tc:
        with (
            tc.tile_pool(name="const", bufs=1) as cpool,
            tc.tile_pool(name="sbuf", bufs=3) as pool,
            tc.tile_pool(name="psum", bufs=2, space="PSUM") as psum,
            tc.tile_pool(name="dram", bufs=1, space="DRAM") as dram,
        ):
            w1_t = cpool.tile([F_IN, ROW1], f16)
            nc.sync.dma_start(out=w1_t[:], in_=w1cat[:])
            w2_t = cpool.tile([F1, ROW2], f32)
            nc.sync.dma_start(out=w2_t[:], in_=w2cat[:])
            b1_t = cpool.tile([P, F1], f32)
            nc.sync.dma_start(out=b1_t[:], in_=b1t[:])
            b2_t = cpool.tile([P, OUT2], f32)
            nc.sync.dma_start(out=b2_t[:], in_=b2t[:])
            iota_t = cpool.tile([P, P], f32)
            nc.sync.dma_start(out=iota_t[:], in_=iota[:])
            ident_t = cpool.tile([P, P], f32)
            nc.sync.dma_start(out=ident_t[:], in_=ident[:])

            t12c = dram.tile([PADN, ROW1], f32)
            t12f = dram.tile([TBLN, ROW1], f32)
            t3c = dram.tile([PADN, ROW2], f32)
            t3f = dram.tile([TBLN, ROW2], f32)

            # ---- phase 0: own slice of t12 = [x@W1 | x@W1 A1s | x@W1 A1d]
            for i in range(NB):
                xTt = pool.tile([F_IN, P], f16, tag="xTt")
                nc.sync.dma_start(out=xTt[:], in_=xT[:, i * P:(i + 1) * P])
                h_ps = psum.tile([P, ROW1], f32, space="PSUM", tag="mmp", bufs=1)
                nc.tensor.matmul(h_ps[:], lhsT=xTt[:], rhs=w1_t[:],
                                 start=True, stop=True)
                h_sb = pool.tile([P, ROW1], f32, tag="hsb")
                nc.vector.tensor_copy(out=h_sb[:], in_=h_ps[:])
                nc.sync.dma_start(out=t12c[:][i * P:(i + 1) * P, :], in_=h_sb[:])

            nc.gpsimd.collective_compute(
                "AllGather", mybir.AluOpType.bypass,
                replica_groups=[list(range(NCORES))],
                ins=[t12c.opt()], outs=[t12f.opt()])

            # ---- layer 1 edge phase; epilogue fuses ELU + layer-2 projection
            def epi1(b, acc):
                r = pool.tile([P, HEADS1], f32, tag="r")
                nc.vector.tensor_scalar(out=r[:], in0=acc[:, F1:F1 + HEADS1],
                                        scalar1=EPS, scalar2=None,
                                        op0=mybir.AluOpType.add)
                nc.vector.reciprocal(out=r[:], in_=r[:])
                o = pool.tile([P, F1], f32, tag="o")
                nc.vector.tensor_tensor(
                    out=_v(o[:], 0, [[OUT1, HEADS1], [1, OUT1]]),
                    in0=_v(acc[:], 0, [[OUT1, HEADS1], [1, OUT1]]),
                    in1=_v(r[:], 0, [[1, HEADS1], [0, OUT1]]),
                    op=mybir.AluOpType.mult)
                nc.vector.tensor_tensor(out=o[:], in0=o[:], in1=b1_t[:],
                                        op=mybir.AluOpType.add)
                # elu(o) = max(o,0) + exp(min(o,0)) - 1
                mn = pool.tile([P, F1], f32, tag="mn")
                nc.vector.tensor_scalar(out=mn[:], in0=o[:], scalar1=0.0,
                                        scalar2=None, op0=mybir.AluOpType.min)
                nc.scalar.activation(mn[:], mn[:],
                                     mybir.ActivationFunctionType.Exp)
                nc.vector.tensor_scalar(out=o[:], in0=o[:], scalar1=0.0,
                                        scalar2=None, op0=mybir.AluOpType.max)
                nc.vector.tensor_tensor(out=o[:], in0=o[:], in1=mn[:],
                                        op=mybir.AluOpType.add)
                nc.vector.tensor_scalar(out=o[:], in0=o[:], scalar1=-1.0,
                                        scalar2=None, op0=mybir.AluOpType.add)
                # t3 rows = elu_out @ [W2 | W2 a2s | W2 a2d]
                oT_ps = psum.tile([P, P], f32, space="PSUM", tag="T", bufs=1)
                nc.tensor.transpose(out=oT_ps[:], in_=o[:], identity=ident_t[:])
                oT = pool.tile([P, F1], f32, tag="oT")
                nc.vector.tensor_copy(out=oT[:], in_=oT_ps[:])
                t3_ps = psum.tile([P, ROW2], f32, space="PSUM", tag="mmp", bufs=1)
                nc.tensor.matmul(t3_ps[:], lhsT=oT[:], rhs=w2_t[:],
                                 start=True, stop=True)
                t3_sb = pool.tile([P, ROW2], f32, tag="t3s")
                nc.vector.tensor_copy(out=t3_sb[:], in_=t3_ps[:])
                nc.sync.dma_start(out=t3c[:][b * P:(b + 1) * P, :], in_=t3_sb[:])

            _edge_phase(nc, (pool, psum), t12f, ROW1, F1, HEADS1,
                        srcg, dstg, dlocg, iota_t, NGRP, TB, epi1)

            nc.gpsimd.collective_compute(
                "AllGather", mybir.AluOpType.bypass,
                replica_groups=[list(range(NCORES))],
                ins=[t3c.opt()], outs=[t3f.opt()])

            # ---- layer 2 edge phase
            def epi2(b, acc):
                r2 = pool.tile([P, 1], f32, tag="r2")
                nc.vector.tensor_scalar(out=r2[:], in0=acc[:, OUT2:OUT2 + 1],
                                        scalar1=EPS, scalar2=None,
                                        op0=mybir.AluOpType.add)
                nc.vector.reciprocal(out=r2[:], in_=r2[:])
                o2 = pool.tile([P, OUT2], f32, tag="o2")
                nc.vector.tensor_tensor(out=o2[:], in0=acc[:, 0:OUT2],
                                        in1=r2[:, 0:1].to_broadcast([P, OUT2]),
                                        op=mybir.AluOpType.mult)
                nc.vector.tensor_tensor(out=o2[:], in0=o2[:], in1=b2_t[:],
                                        op=mybir.AluOpType.add)
                o2h = pool.tile([P, OUT2], f16, tag="o2h")
                nc.vector.tensor_copy(out=o2h[:], in_=o2[:])
                nc.sync.dma_start(out=oout[b * P:(b + 1) * P, :], in_=o2h[:])

            _edge_phase(nc, (pool, psum), t3f, ROW2, OUT2, HEADS2,
                        srcg, dstg, dlocg, iota_t, NGRP, TB, epi2)

    nc.compile()
    _split_excess_waits(nc)
    return nc


# ---------------------------------------------------------------------------
# launcher: AOT-compile the PJRT wrapper once, reuse across calls

def _make_runner(nc):
    import jax
    from jax.sharding import Mesh, PartitionSpec
    from jax.experimental.shard_map import shard_map
    from concourse.bass2jax import (install_neuronx_cc_hook, _bass_exec_p,
                                    partition_id_tensor)

    install_neuronx_cc_hook()
    partition_name = nc.partition_id_tensor.name if nc.partition_id_tensor else None
    in_names, out_names, out_avals = [], [], []
    for alloc in nc.m.functions[0].allocations:
        if not isinstance(alloc, mybir.MemoryLocationSet):
            continue
        name = alloc.memorylocations[0].name
        if alloc.kind == "ExternalInput":
            if name != partition_name:
                in_names.append(name)
        elif alloc.kind == "ExternalOutput":
            out_names.append(name)
            out_avals.append(jax.core.ShapedArray(
                tuple(alloc.tensor_shape), mybir.dt.np(alloc.dtype)))
    n_params = len(in_names)
    all_names = list(in_names) + list(out_names)
    if partition_name is not None:
        all_names.append(partition_name)
    donate = tuple(range(n_params, n_params + len(out_names)))

    def _body(*args):
        operands = list(args)
        if partition_name is not None:
            operands.append(partition_id_tensor())
        return tuple(_bass_exec_p.bind(
            *operands, out_avals=tuple(out_avals), in_names=tuple(all_names),
            out_names=tuple(out_names), lowering_input_output_aliases=(),
            sim_require_finite=True, sim_require_nnan=True, nc=nc))

    devices = jax.devices()[:NCORES]
    mesh = Mesh(np.asarray(devices), ("core",))
    nio = n_params + len(out_names)
    sharded = jax.jit(
        shard_map(_body, mesh=mesh, in_specs=(PartitionSpec("core"),) * nio,
                  out_specs=(PartitionSpec("core"),) * len(out_names),
                  check_rep=False),
        donate_argnums=donate, keep_unused=True)
    in_structs = []
    for alloc in nc.m.functions[0].allocations:
        if not isinstance(alloc, mybir.MemoryLocationSet):
            continue
        if alloc.memorylocations[0].name in in_names:
            shp = tuple(alloc.tensor_shape)
            in_structs.append(jax.ShapeDtypeStruct(
                (NCORES * shp[0],) + shp[1:], mybir.dt.np(alloc.dtype)))
    out_structs = [jax.ShapeDtypeStruct((NCORES * a.shape[0],) + a.shape[1:],
                                        a.dtype) for a in out_avals]
    compiled = sharded.lower(*in_structs, *out_structs).compile()
    return {
        "compiled": compiled,
        "in_names": in_names,
        "out_names": out_names,
        "shardings": dict(zip(in_names + out_names,
                              compiled.input_shardings[0])),
        "out_structs": [(tuple(s.shape), s.dtype) for s in out_structs],
        "in_structs": [(tuple(s.shape), s.dtype) for s in in_structs],
    }


_RUNNERS = {}


def _get_runner(TB, NGRP, warm=False):
    key = (TB, NGRP)
    if key not in _RUNNERS:
        nc = _build_kernel(TB, NGRP)
        runner = _make_runner(nc)
        if warm:
            import jax
            ins = [np.zeros(s, d) for s, d in runner["in_structs"]]
            outs = [np.zeros(s, d) for s, d in runner["out_structs"]]
            jax.block_until_ready(runner["compiled"](*ins, *outs))
        _RUNNERS[key] = runner
    return _RUNNERS[key]


# ---------------------------------------------------------------------------
# host-side edge prep (vectorized)

def _prep_edges(src32, dst32, TB_hint):
    """Pack edges into per-core [NGRP, P, G] grids (concatenated on axis 0),
    writing the device layout directly. Within-block slot order is arbitrary
    (the on-device scatter-sum is order-invariant).
    Returns (srcg, dstg, dlocg, TB, NGRP)."""
    E = len(dst32)
    ci, ld = np.divmod(dst32, NPC)     # owning core, local dst within slice
    blk_l = ld >> 7
    pos = ld & 127                     # dst slot within block (= dloc)
    blk_g = ci * NB + blk_l
    cnt = np.bincount(blk_g, minlength=NCORES * NB)
    TB = max(int(-(-cnt.max() // P)), 1, TB_hint)
    starts = np.zeros(NCORES * NB, np.int64)
    np.cumsum(cnt[:-1], out=starts[1:])
    starts32 = starts.astype(np.int32)
    order = np.argsort(blk_g, kind='stable')
    rank = np.empty(E, np.int32)       # running index within the dst block
    rank[order] = np.arange(E, dtype=np.int32) - starts32[blk_g[order]]
    gtile = blk_l * TB + (rank >> 7)   # tile id within core grid
    part = rank & 127                  # partition (edge slot within tile)
    q = gtile >> 4                     # group id (G == 16)
    tg = gtile & 15
    ntiles = NB * TB
    NGRP = -(-ntiles // G)
    nslots = NGRP * P * G
    flat = ci * nslots + (((q << 7) + part) << 4) + tg
    srcg = np.zeros(NCORES * nslots, np.uint16)
    dstg = np.zeros(NCORES * nslots, np.uint16)
    dlocg = np.full(NCORES * nslots, DLOC_PAD, np.uint8)
    sq, sr = np.divmod(src32, NPC)
    srcg[flat] = (sq * PADN + sr).astype(np.uint16)
    dstg[flat] = (ci * PADN + ld).astype(np.uint16)
    dlocg[flat] = pos.astype(np.uint8)
    shp = (NCORES * NGRP, P, G)
    return srcg.reshape(shp), dstg.reshape(shp), dlocg.reshape(shp), TB, NGRP


def kernel(x, edge_index, W1, a_src1, a_dst1, b1, W2, a_src2, a_dst2, b2):
    import jax
    x = np.asarray(x, np.float32)
    assert x.shape == (N, F_IN), f"unexpected x shape {x.shape}"
    default = _RUNNERS.get((TB_DEFAULT, -(-(NB * TB_DEFAULT) // G)))

    # pack x + weights first and start their transfers (overlaps edge prep)
    xT = np.zeros((NCORES, F_IN, PADN), np.float16)
    for k in range(NCORES):
        xT[k, :, :NPC] = x[k * NPC:(k + 1) * NPC].T
    xT = xT.reshape(NCORES * F_IN, PADN)
    W1 = np.asarray(W1, np.float32)
    A1s = np.zeros((F1, HEADS1), np.float32)
    A1d = np.zeros((F1, HEADS1), np.float32)
    for h in range(HEADS1):
        A1s[h * OUT1:(h + 1) * OUT1, h] = np.asarray(a_src1, np.float32)[h]
        A1d[h * OUT1:(h + 1) * OUT1, h] = np.asarray(a_dst1, np.float32)[h]
    w1cat = np.concatenate([W1, W1 @ A1s, W1 @ A1d], axis=1)  # [F_IN, 136]
    W2 = np.asarray(W2, np.float32)
    w2cat = np.concatenate(
        [W2, W2 @ np.asarray(a_src2, np.float32).reshape(OUT2, 1),
         W2 @ np.asarray(a_dst2, np.float32).reshape(OUT2, 1)], axis=1)

    def rep(a):  # replicate a per-core constant 8x along axis 0
        return np.tile(a, (NCORES,) + (1,) * (a.ndim - 1))

    arrays = {
        "xT": xT,
        "w1cat": rep(w1cat.astype(np.float16)),
        "w2cat": rep(w2cat),
        "b1t": rep(np.tile(np.asarray(b1, np.float32)[None, :], (P, 1))),
        "b2t": rep(np.tile(np.asarray(b2, np.float32)[None, :], (P, 1))),
        "iota": rep(np.tile(np.arange(P, dtype=np.float32)[None, :], (P, 1))),
        "ident": rep(np.eye(P, dtype=np.float32)),
    }
    if default is not None:
        sh = default["shardings"]
        arrays = {k: jax.device_put(v, sh[k]) for k, v in arrays.items()}
        outs = [jax.device_put(np.zeros(s, d), sh[nm]) for (s, d), nm in
                zip(default["out_structs"], default["out_names"])]

    # edge prep on CPU while the above transfers stream in
    e0 = np.asarray(edge_index[0])
    E = e0.shape[0]
    src32 = np.empty(E + N, np.int32)
    src32[:E] = e0
    src32[E:] = np.arange(N, dtype=np.int32)   # self loops
    dst32 = np.empty(E + N, np.int32)
    dst32[:E] = np.asarray(edge_index[1])
    dst32[E:] = src32[E:]
    srcg, dstg, dlocg, TB, NGRP = _prep_edges(src32, dst32, TB_DEFAULT)
    runner = _get_runner(TB, NGRP)
    if runner is not default:
        arrays = {k: np.asarray(v) for k, v in arrays.items()}
        outs = [np.zeros(s, d) for s, d in runner["out_structs"]]
        grids = {"srcg": srcg, "dstg": dstg, "dlocg": dlocg}
    else:
        sh = default["shardings"]
        grids = {"srcg": jax.device_put(srcg, sh["srcg"]),
                 "dstg": jax.device_put(dstg, sh["dstg"]),
                 "dlocg": jax.device_put(dlocg, sh["dlocg"])}
    arrays.update(grids)

    ins = [arrays[nm] for nm in runner["in_names"]]
    res = runner["compiled"](*ins, *outs)
    oidx = runner["out_names"].index("oout")
    oo = np.asarray(res[oidx]).reshape(NCORES, PADN, OUT2)
    return oo[:, :NPC, :].astype(np.float32).reshape(N, OUT2)


# precompile + warm at import (shapes are static for this problem)
_DEFAULT_NGRP = -(-(NB * TB_DEFAULT) // G)
try:
    _get_runner(TB_DEFAULT, _DEFAULT_NGRP, warm=True)
except Exception:
    _RUNNERS.clear()
